# revision 2
# baseline (speedup 1.0000x reference)
"""Trainium2 Bass kernel for nn_CausalGraphLayer (gnn_message_passing).

Math: out[b,n,t,c] = tanh( sum_k w[c,n,k] * z[b, idx[n,k], t, c] )
      w[c,n,k] = (sum_nb coeff[c,nb] * bases[nb,n,k]) * adj[n,k]

Decomposition used here (v2 — coeff folded into z on host):
  A_nb[m,n]      = sum_k bases[nb,n,k]*adj[n,k]*[idx[n,k]==m]  (host, dense)
  z4[nb,m,f]     = z[m,f] * coeff[c(f),nb]                      (host)
  out[n,f]       = tanh( sum_{nb,m} A_nb[m,n] * z4[nb,m,f] )    (PE + ACT)

Folding coeff into z makes the whole kernel ONE PSUM-accumulated matmul
chain over the joint (nb,m) contraction (32 chunks of 128): the baseline's
per-basis PSUM tiles plus a DVE combine (mul + 2 adds, ~137us vector time)
disappear entirely; ACT applies tanh straight from PSUM. Everything is
bf16 (PE rate is 1 row/cycle for bf16 and fp32r alike, so bf16 costs no
PE time and halves DMA+SBUF). fp8 DoubleRow would halve PE time but its
~5e-2 quantization error exceeds the 2e-2 tolerance (measured).

Sharding: 8 cores = 4 batches x 2 destination-node halves (unchanged).
DMA per core: z4 32MB + A 4MB in, out 4MB — ~112us at HBM rate, under the
~218us of bf16 matmul, so the kernel is PE-bound end to end.
"""

import sys

import numpy as np

B, N, T, C = 4, 1024, 64, 64
K_CURR, MAX_K, NUM_BASES = 16, 32, 4
TC = T * C  # 4096
HALF = N // 2  # 512 destination nodes per core
NI = HALF // 128  # 4 output row-chunks
MCHUNK = N // 128  # 8 source chunks
CH = NUM_BASES * MCHUNK  # 32 joint (nb, m) contraction chunks
CB = 8  # A load pieces (4 chunks each)
FREE = 512  # matmul moving free dim (one PSUM bank, fp32)
FJ = TC // FREE  # 8 free slabs
# fj=0 arrives in progressively larger pieces so the first matmul can
# start after ~256KB instead of a full 4MB slab.
Z0_SPLIT = (2, 2, 4, 8, 16)

_CACHE = {}


def _import_concourse():
    try:
        import concourse.bass  # noqa: F401
    except ImportError:
        for p in ("/opt/trn_rl_repo", "/root/.axon_site/_ro/trn_rl_repo"):
            if p not in sys.path:
                sys.path.append(p)
        import concourse.bass  # noqa: F401


def _split_multi_waits(nc):
    """Split multi-sem waits into single-wait NOPs.

    The TPB ISA has one wait slot per instruction; the walrus build in this
    container errors with "Too many sync wait commands" on instructions
    carrying more than one SyncWait (Tile's tail drain does). Splitting into
    preceding same-engine NOPs is semantics-preserving: engine queues are
    FIFO and semaphores are monotone.
    """
    import concourse.mybir as mybir

    counter = [0]
    for fn in nc.m.functions:
        for bb in fn.blocks:
            new_insts = []
            changed = False
            for inst in bb.instructions:
                si = inst.sync_info
                if si is not None and si.on_wait and len(si.on_wait) > 1:
                    waits = list(si.on_wait)
                    for w in waits[:-1]:
                        counter[0] += 1
                        nop = mybir.InstNoOp(
                            name=f"WSPLIT-{counter[0]}", engine=inst.engine
                        )
                        nop.sync_info = mybir.SyncInfo(on_wait=[w], on_update=[])
                        new_insts.append(nop)
                    inst.sync_info = mybir.SyncInfo(
                        on_wait=[waits[-1]], on_update=list(si.on_update)
                    )
                    changed = True
                new_insts.append(inst)
            if changed:
                bb.instructions = new_insts
    return nc


def _build_program():
    import concourse.bass as bass
    import concourse.mybir as mybir
    from concourse import tile

    f32 = mybir.dt.float32
    bf16 = mybir.dt.bfloat16

    nc = bass.Bass("TRN2", target_bir_lowering=False, debug=False)
    # z4: coeff-folded z, [fj, p, ch, f]; per-partition data for one slab is
    # a contiguous 32KB run, so whole-slab loads hit full DMA bandwidth.
    z_d = nc.dram_tensor("z", [FJ, 128, CH, FREE], bf16, kind="ExternalInput")
    # A: [cb, p, cw, ni, n] with ch = cb*4+cw; one cb piece feeds all 4 ni.
    a_d = nc.dram_tensor("a", [CB, 128, 4, NI, 128], bf16, kind="ExternalInput")
    o_d = nc.dram_tensor("out", [HALF, TC], bf16, kind="ExternalOutput")

    with tile.TileContext(nc) as tc:
        with (
            tc.tile_pool(name="z0p", bufs=1) as z0p,
            tc.tile_pool(name="zp", bufs=2) as zp,
            tc.tile_pool(name="apool", bufs=1) as apool,
            tc.tile_pool(name="psum", bufs=4, space="PSUM") as psp,
            tc.tile_pool(name="outp", bufs=4) as outp,
        ):
            # A pieces on the sync (SP HWDGE) ring; z on SWDGE keeps the two
            # descriptor streams independent (the SDMA engines round-robin).
            a_ts = []
            for cb in range(CB):
                a_t = apool.tile([128, 4, NI, 128], bf16, tag=f"a{cb}")
                nc.sync.dma_start(out=a_t[:], in_=a_d[cb])
                a_ts.append(a_t)

            # fj=0 in pieces (first matmul after ~3us), rest whole slabs.
            z0_ts = []
            s = 0
            for i, w in enumerate(Z0_SPLIT):
                z0_t = z0p.tile([128, w, FREE], bf16, tag=f"z0-{i}")
                nc.gpsimd.dma_start(out=z0_t[:], in_=z_d[0][:, s : s + w, :])
                z0_ts.append((s, s + w, z0_t))
                s += w

            def rhs0(ch):
                for s, e, t in z0_ts:
                    if s <= ch < e:
                        return t[:, ch - s, :]
                raise AssertionError

            for fj in range(FJ):
                if fj > 0:
                    z_t = zp.tile([128, CH, FREE], bf16, tag="z")
                    nc.gpsimd.dma_start(out=z_t[:], in_=z_d[fj])
                for ni in range(NI):
                    ps = psp.tile([128, FREE], f32, tag="ps")
                    for ch in range(CH):
                        rhs = rhs0(ch) if fj == 0 else z_t[:, ch, :]
                        nc.tensor.matmul(
                            ps[:],
                            a_ts[ch // 4][:, ch % 4, ni, :],
                            rhs,
                            start=(ch == 0),
                            stop=(ch == CH - 1),
                        )
                    out_t = outp.tile([128, FREE], bf16, tag="o")
                    nc.scalar.activation(
                        out_t[:], ps[:], mybir.ActivationFunctionType.Tanh
                    )
                    # Stores issue from the ACT ring (2nd HWDGE ring), so the
                    # wait-on-tanh never blocks the SP ring carrying A loads.
                    nc.scalar.dma_start(
                        out=o_d[bass.ts(ni, 128), bass.ts(fj, FREE)], in_=out_t[:]
                    )

    _split_multi_waits(nc)
    return nc


def _host_prep(z, neighbor_indices, adjacency, basis_weights, channel_coeffs):
    """Build per-core input maps (bf16, coeff folded into z)."""
    import ml_dtypes

    bf16 = ml_dtypes.bfloat16
    z = np.asarray(z, dtype=np.float32)
    idx = np.asarray(neighbor_indices)
    k = idx.shape[1]
    if k > adjacency.shape[1]:
        idx = idx[:, : adjacency.shape[1]]
        k = adjacency.shape[1]
    adj = np.asarray(adjacency, dtype=np.float32)[:, :k]
    bases = np.asarray(basis_weights, dtype=np.float32)[:, :, :k]
    coeff = np.asarray(channel_coeffs, dtype=np.float32)

    abases = bases * adj[None, :, :]  # (NB, N, k)
    # cf[nb, f] = coeff[f % C, nb] over f = t*C + c
    cf = np.tile(coeff.T[:, None, :], (1, T, 1)).reshape(NUM_BASES, TC)
    cfr = cf.reshape(NUM_BASES, FJ, FREE)

    # z4 per batch (shared by the two half-N cores of that batch):
    # [fj, p, ch=(nb,mc), f] bf16
    z4s = []
    for b in range(B):
        zb = z[b].reshape(MCHUNK, 128, FJ, FREE)  # [mc, p, fj, f]
        z4 = zb[None] * cfr[:, None, None]  # [nb, mc, p, fj, f]
        z4s.append(
            np.ascontiguousarray(z4.transpose(3, 2, 0, 1, 4))
            .reshape(FJ, 128, CH, FREE)
            .astype(bf16)
        )

    in_maps = []
    for core in range(8):
        b, h = divmod(core, 2)
        rows = slice(h * HALF, (h + 1) * HALF)
        idx_h = idx[rows]  # (HALF, k)
        # a_t[nb, m, nl] = sum_k abases[nb, n, k] over idx[n,k]==m
        a_t = np.zeros((NUM_BASES, N, HALF), dtype=np.float32)
        cols = np.repeat(np.arange(HALF), k)
        flat_idx = idx_h.ravel()
        for nb in range(NUM_BASES):
            np.add.at(a_t[nb], (flat_idx, cols), abases[nb, rows].ravel())
        # [cb, p, cw, ni, n] with ch = nb*MCHUNK + mc = cb*4 + cw
        a_seq = a_t.reshape(NUM_BASES, MCHUNK, 128, NI, 128).reshape(
            CH, 128, NI, 128
        )
        a_pack = np.ascontiguousarray(
            a_seq.reshape(CB, 4, 128, NI, 128).transpose(0, 2, 1, 3, 4)
        ).astype(bf16)
        in_maps.append({"z": z4s[b], "a": a_pack})
    return in_maps


def _get_program():
    key = "nc"
    if key not in _CACHE:
        _import_concourse()
        _CACHE[key] = _build_program()
    return _CACHE[key]


def run_on_hw(in_maps, **kwargs):
    from concourse.bass_utils import run_bass_kernel_spmd

    nc = _get_program()
    return run_bass_kernel_spmd(nc, in_maps, core_ids=list(range(8)), **kwargs)


def kernel(z, neighbor_indices, adjacency, basis_weights, channel_coeffs):
    _import_concourse()
    in_maps = _host_prep(z, neighbor_indices, adjacency, basis_weights, channel_coeffs)
    res = run_on_hw(in_maps)
    out = np.empty((B, N, T, C), dtype=np.float32)
    for core in range(8):
        b, h = divmod(core, 2)
        out[b, h * HALF : (h + 1) * HALF] = (
            res.results[core]["out"].astype(np.float32).reshape(HALF, T, C)
        )
    return out


# revision 8
# speedup vs baseline: 1.1102x; 1.1102x over previous
"""Trainium2 Bass kernel for nn_CausalGraphLayer (gnn_message_passing).

Math: out[b,n,t,c] = tanh( sum_k w[c,n,k] * z[b, idx[n,k], t, c] )
      w[c,n,k] = (sum_nb coeff[c,nb] * bases[nb,n,k]) * adj[n,k]

Decomposition (v3 — coeff folded into z, expansion on-chip):
  A_nb[m,n]   = sum_k bases[nb,n,k]*adj[n,k]*[idx[n,k]==m]   (host, dense)
  z4[nb,m,f]  = z[m,f] * coeff[c(f),nb]                       (DVE, on-chip)
  out[n,f]    = tanh( sum_{nb,m} A_nb[m,n] * z4[nb,m,f] )     (PE + ACT)

Folding coeff into z makes the kernel ONE PSUM-accumulated matmul chain
over the joint (nb,m) contraction (32 chunks of 128): the baseline's
per-basis PSUM tiles plus a DVE combine (mul + 2 adds, ~137us vector
time) disappear; ACT applies tanh straight from PSUM. The 4x expansion
of z happens on the otherwise-idle DVE (~9us/slab vs 27us/slab of PE),
so z crosses HBM exactly once — shipping pre-expanded z4 from HBM
(32MB/core) starved the PE for ~35us at startup since the first 3 slabs
plus A exceed HBM bandwidth; expanding on-chip cuts input DMA to 12.5MB.

Everything is bf16: PE rate is 1 row/cycle for bf16 and fp32r alike, so
bf16 costs no PE time and halves DMA+SBUF. fp8 DoubleRow would halve PE
time but its ~5e-2 quantization error exceeds the 2e-2 tolerance
(measured on the actual inputs).

Sharding: 8 cores = 4 batches x 2 destination-node halves.
"""

import sys

import numpy as np

B, N, T, C = 4, 1024, 64, 64
K_CURR, MAX_K, NUM_BASES = 16, 32, 4
TC = T * C  # 4096
HALF = N // 2  # 512 destination nodes per core
NI = HALF // 128  # 4 output row-chunks
MC = N // 128  # 8 source chunks
CH = NUM_BASES * MC  # 32 joint (mc, nb) contraction chunks
FREE = 512  # matmul moving free dim (one PSUM bank, fp32)
FJ = TC // FREE  # 8 free slabs

_CACHE = {}


def _import_concourse():
    try:
        import concourse.bass  # noqa: F401
    except ImportError:
        for p in ("/opt/trn_rl_repo", "/root/.axon_site/_ro/trn_rl_repo"):
            if p not in sys.path:
                sys.path.append(p)
        import concourse.bass  # noqa: F401


def _split_multi_waits(nc):
    """Split multi-sem waits into single-wait NOPs.

    The TPB ISA has one wait slot per instruction; the walrus build in this
    container errors with "Too many sync wait commands" on instructions
    carrying more than one SyncWait (Tile's tail drain does). Splitting into
    preceding same-engine NOPs is semantics-preserving: engine queues are
    FIFO and semaphores are monotone.
    """
    import concourse.mybir as mybir

    counter = [0]
    for fn in nc.m.functions:
        for bb in fn.blocks:
            new_insts = []
            changed = False
            for inst in bb.instructions:
                si = inst.sync_info
                if si is not None and si.on_wait and len(si.on_wait) > 1:
                    waits = list(si.on_wait)
                    for w in waits[:-1]:
                        counter[0] += 1
                        nop = mybir.InstNoOp(
                            name=f"WSPLIT-{counter[0]}", engine=inst.engine
                        )
                        nop.sync_info = mybir.SyncInfo(on_wait=[w], on_update=[])
                        new_insts.append(nop)
                    inst.sync_info = mybir.SyncInfo(
                        on_wait=[waits[-1]], on_update=list(si.on_update)
                    )
                    changed = True
                new_insts.append(inst)
            if changed:
                bb.instructions = new_insts
    return nc


def _build_program():
    import concourse.bass as bass
    import concourse.mybir as mybir
    from concourse import tile

    f32 = mybir.dt.float32
    bf16 = mybir.dt.bfloat16

    nc = bass.Bass("TRN2", target_bir_lowering=False, debug=False)
    # z: [fj, p, mc, f] — per-partition data for one slab is a contiguous
    # 8KB run, so whole-slab (1MB) loads run near full DMA bandwidth.
    z_d = nc.dram_tensor("z", [FJ, 128, MC, FREE], bf16, kind="ExternalInput")
    # A: [mc, p, nb, ni, n]; one 512KB piece per source chunk mc.
    a_d = nc.dram_tensor("a", [MC, 128, NUM_BASES, NI, 128], bf16, kind="ExternalInput")
    # bc[p, nb, f] = coeff[f % C, nb], partition-replicated on host (512KB).
    bc_d = nc.dram_tensor("bc", [128, NUM_BASES, FREE], bf16, kind="ExternalInput")
    o_d = nc.dram_tensor("out", [HALF, TC], bf16, kind="ExternalOutput")

    with tile.TileContext(nc) as tc:
        with (
            tc.tile_pool(name="z0p", bufs=1) as z0p,
            tc.tile_pool(name="zp", bufs=3) as zp,
            tc.tile_pool(name="z4p", bufs=2) as z4p,
            tc.tile_pool(name="apool", bufs=1) as apool,
            tc.tile_pool(name="bcp", bufs=1) as bcp,
            # 4 ps tags x 2 bufs x 2KB = all 8 PSUM banks (bufs is per-tag)
            tc.tile_pool(name="psum", bufs=2, space="PSUM") as psp,
            tc.tile_pool(name="outp", bufs=4) as outp,
        ):
            # bc first on the scalar ring — every DVE mul needs it, and the
            # sync ring starts on A pieces concurrently.
            bc_t = bcp.tile([128, NUM_BASES, FREE], bf16)
            nc.scalar.dma_start(out=bc_t[:], in_=bc_d.ap())

            # A pieces alternate between the two HWDGE rings (SP + ACT) so
            # all of A lands in ~half the time; z is on SWDGE (gpsimd).
            # Stores don't need the ACT ring until ~12us in.
            a_ts = []
            for mc in range(MC):
                a_t = apool.tile([128, NUM_BASES, NI, 128], bf16, tag=f"a{mc}")
                eng = nc.sync if mc % 2 == 0 else nc.scalar
                eng.dma_start(out=a_t[:], in_=a_d[mc])
                a_ts.append(a_t)

            # fj=0 in per-mc pieces so the first DVE mul + matmul can start
            # after ~128KB of z instead of a full slab.
            z0_ts = []
            for mc in range(MC):
                z0_t = z0p.tile([128, FREE], bf16, tag=f"z0-{mc}")
                nc.gpsimd.dma_start(out=z0_t[:], in_=z_d[0][:, mc, :])
                z0_ts.append(z0_t)

            for fj in range(FJ):
                if fj > 0:
                    z_t = zp.tile([128, MC, FREE], bf16, tag="z")
                    nc.gpsimd.dma_start(out=z_t[:], in_=z_d[fj])
                # DVE expansion: z4[mc][:, nb, :] = z_slab[mc] * bc[nb]
                z4_ts = []
                for mc in range(MC):
                    z4_t = z4p.tile([128, NUM_BASES, FREE], bf16, tag=f"z4-{mc}")
                    src = z0_ts[mc][:] if fj == 0 else z_t[:, mc, :]
                    for nb in range(NUM_BASES):
                        nc.vector.tensor_mul(z4_t[:, nb, :], src, bc_t[:, nb, :])
                    z4_ts.append(z4_t)
                # ni innermost: 4 parallel PSUM chains consume each z4 chunk
                # right after its DVE mul, so the fj=0 warmup tracks the DVE
                # expansion instead of trailing a full slab behind it.
                ps_ts = [
                    psp.tile([128, FREE], f32, tag=f"ps{ni}", name=f"ps{ni}")
                    for ni in range(NI)
                ]
                for ch in range(CH):
                    mc, nb = divmod(ch, NUM_BASES)
                    for ni in range(NI):
                        nc.tensor.matmul(
                            ps_ts[ni][:],
                            a_ts[mc][:, nb, ni, :],
                            z4_ts[mc][:, nb, :],
                            start=(ch == 0),
                            stop=(ch == CH - 1),
                        )
                for ni in range(NI):
                    out_t = outp.tile([128, FREE], bf16, tag="o")
                    nc.scalar.activation(
                        out_t[:], ps_ts[ni][:], mybir.ActivationFunctionType.Tanh
                    )
                    # Stores issue from the ACT ring (2nd HWDGE ring), so the
                    # wait-on-tanh never blocks the SP ring carrying A loads.
                    nc.scalar.dma_start(
                        out=o_d[bass.ts(ni, 128), bass.ts(fj, FREE)], in_=out_t[:]
                    )

    _split_multi_waits(nc)
    return nc


def _host_prep(z, neighbor_indices, adjacency, basis_weights, channel_coeffs):
    """Build per-core input maps (bf16; z shipped once, coeff separate)."""
    import ml_dtypes

    bf16 = ml_dtypes.bfloat16
    z = np.asarray(z, dtype=np.float32)
    idx = np.asarray(neighbor_indices)
    k = idx.shape[1]
    if k > adjacency.shape[1]:
        idx = idx[:, : adjacency.shape[1]]
        k = adjacency.shape[1]
    adj = np.asarray(adjacency, dtype=np.float32)[:, :k]
    bases = np.asarray(basis_weights, dtype=np.float32)[:, :, :k]
    coeff = np.asarray(channel_coeffs, dtype=np.float32)

    abases = bases * adj[None, :, :]  # (NB, N, k)

    # bc[p, nb, f] = coeff[f % C, nb] (identical for every f-slab)
    cfslab = np.tile(coeff.T[:, None, :], (1, FREE // C, 1)).reshape(NUM_BASES, FREE)
    bc = np.ascontiguousarray(
        np.broadcast_to(cfslab[None], (128, NUM_BASES, FREE))
    ).astype(bf16)

    # z_re[fj, p, mc, f] per batch (shared by that batch's two cores)
    z_res = []
    for b in range(B):
        z_res.append(
            np.ascontiguousarray(
                z[b].reshape(MC, 128, FJ, FREE).transpose(2, 1, 0, 3)
            ).astype(bf16)
        )

    in_maps = []
    for core in range(8):
        b, h = divmod(core, 2)
        rows = slice(h * HALF, (h + 1) * HALF)
        idx_h = idx[rows]  # (HALF, k)
        # a_t[nb, m, nl] = sum_k abases[nb, n, k] over idx[n,k]==m
        a_t = np.zeros((NUM_BASES, N, HALF), dtype=np.float32)
        cols = np.repeat(np.arange(HALF), k)
        flat_idx = idx_h.ravel()
        for nb in range(NUM_BASES):
            np.add.at(a_t[nb], (flat_idx, cols), abases[nb, rows].ravel())
        # [mc, p, nb, ni, n]
        a_pack = np.ascontiguousarray(
            a_t.reshape(NUM_BASES, MC, 128, NI, 128).transpose(1, 2, 0, 3, 4)
        ).astype(bf16)
        in_maps.append({"z": z_res[b], "a": a_pack, "bc": bc})
    return in_maps


def _get_program():
    key = "nc"
    if key not in _CACHE:
        _import_concourse()
        _CACHE[key] = _build_program()
    return _CACHE[key]


def run_on_hw(in_maps, **kwargs):
    from concourse.bass_utils import run_bass_kernel_spmd

    nc = _get_program()
    return run_bass_kernel_spmd(nc, in_maps, core_ids=list(range(8)), **kwargs)


def kernel(z, neighbor_indices, adjacency, basis_weights, channel_coeffs):
    _import_concourse()
    in_maps = _host_prep(z, neighbor_indices, adjacency, basis_weights, channel_coeffs)
    res = run_on_hw(in_maps)
    out = np.empty((B, N, T, C), dtype=np.float32)
    for core in range(8):
        b, h = divmod(core, 2)
        out[b, h * HALF : (h + 1) * HALF] = (
            res.results[core]["out"].astype(np.float32).reshape(HALF, T, C)
        )
    return out
